# revision 5
# baseline (speedup 1.0000x reference)
"""CNOT permutation kernel for Trainium2 (Bass), 8-core data parallel.

Problem (hardcoded from spec): state (16, 2**24) f32, control=3, target=10,
num_qubits=24.  With c2 = 24-3-1 = 20 and t2 = 24-10-1 = 13:

    out[b, j] = state[b, j ^ (1<<13)]  if (j >> 20) & 1 else state[b, j]

Pure data movement.  Viewing the per-core shard flat (row stride 2**24 is a
multiple of the 2**21 control-bit period, so both rows fuse) as
[blk:16, ctrl:2, c:64, d:2, e:8192]:

    out[blk, 0, c, d, e] = in[blk, 0, c, d, e]      (identity half)
    out[blk, 1, c, d, e] = in[blk, 1, c, 1-d, e]    (swap 8192-elem chunk pairs)

Sharding: batch axis, 2 rows per core (pure data parallel).

Device kernels are HBM->SBUF->HBM bounces (direct DRAM->DRAM DMA serializes
read+write inside each SDMA engine; split load/store descriptors stream at
the SBUF-AXI fabric rate ~435 GB/s/core).  Loads issue on the Sync HWDGE
ring, stores on the Scalar ring; the 16 SDMA engines round-robin between
rings at packet granularity so both HBM directions stay busy.

Pipeline structure (this is where the previous version lost ~60 us/core):
the work is cut into variable-size units -- small at the head (first store
starts after a ~1 MiB load instead of 4 MiB) and geometrically shrinking at
the tail (the final load->store chain, which nothing can overlap, is 0.25
MiB instead of 4 MiB).  Units are one load + one CONTIGUOUS store each: the
chunk-pair swap is folded into the load AP (middle dim with stride -TBIT),
so HBM writes are fully sequential.  SBUF ring of NSLOT 32 KiB/partition
slots, two semaphores (RAW: store waits its unit's load; WAR: load waits
the store that previously used its slot).

1. In-place (default): a single DRAM tensor, pre-initialized with the input
   shard by donating it as the PJRT output buffer (the same donation
   mechanism run_bass_via_pjrt uses for its zero-filled outputs).  Only the
   control-bit=1 half is touched: 64 MiB read + 64 MiB write per core.  The
   identity half is never moved at all.
2. Full-copy (fallback): separate in/out tensors, 128+128 MiB per core.

kernel() runs the in-place path and sample-checks the permutation against
the host input; if the donation aliasing ever fails to hold (output buffer
not seeded with the input), it falls back to the full-copy path.
"""

import numpy as np

import concourse.bass as bass
import concourse.mybir as mybir
from concourse.bass_utils import run_bass_kernel_spmd

NUM_QUBITS = 24
DIM = 1 << NUM_QUBITS
BATCH = 16
N_CORES = 8
ROWS = BATCH // N_CORES  # 2 rows per core
C2 = NUM_QUBITS - 3 - 1  # 20
T2 = NUM_QUBITS - 10 - 1  # 13
CBIT = 1 << C2  # 1048576 elements (4 MiB)
TBIT = 1 << T2  # 8192 elements (32 KiB)
BLK = 2 * CBIT  # control-bit period
NBLK = ROWS * DIM // BLK  # 16 blocks in the fused per-core space

P = 128
FREE = CBIT // P  # 8192 elems/partition: slot is [128, 8192] f32 = 4 MiB
NSLOT = 6
PAIRS = CBIT // (2 * TBIT)  # 64 chunk pairs per 4 MiB region

# Unit splits along the within-chunk e axis (elements, each unit spans all
# 128 partitions): first region ramps up, last region ramps down so the
# final un-overlappable load->store chain is tiny.
HEAD_SPLIT = [2048, 2048, 4096]
TAIL_SPLIT = [4096, 2048, 1024, 512, 512]

_cache = {}


def _units(inplace):
    """Yield (kind, region_base, e0, w): the unit covers the e in [e0,e0+w)
    slice of all 128 TBIT-chunks of its 4 MiB region.  kind 's'=swap,
    'i'=identity."""
    units = []
    for b in range(NBLK):
        if not inplace:
            units.append(("i", b * BLK, 0, TBIT))
        units.append(("s", b * BLK + CBIT, 0, TBIT))
    first, last = units[0], units[-1]
    head = [(first[0], first[1], sum(HEAD_SPLIT[:k]), w)
            for k, w in enumerate(HEAD_SPLIT)]
    tail = [(last[0], last[1], sum(TAIL_SPLIT[:k]), w)
            for k, w in enumerate(TAIL_SPLIT)]
    return head + units[1:-1] + tail


def _emit_bounce(nc, src, dst, units):
    """Loads on sync / stores on scalar, NSLOT-deep pipeline over units."""
    n = len(units)
    with (
        nc.sbuf_tensor("tiles", [P, NSLOT * FREE], mybir.dt.float32) as tiles,
        nc.semaphore("load_sem") as load_sem,
        nc.semaphore("store_sem") as store_sem,
        nc.Block() as block,
    ):

        def tile_view(i, w):
            s = (i % NSLOT) * FREE
            return tiles[:, s : s + w]

        @block.sync
        def _(sync):
            for i, (kind, base, e0, w) in enumerate(units):
                if i >= NSLOT:
                    sync.wait_ge(store_sem, 16 * (i - NSLOT + 1))
                if kind == "s":
                    # partition p = (c, j) reads chunk (c, 1-j)'s e-slice
                    in_ap = bass.AP(
                        src,
                        base + TBIT + e0,
                        [[2 * TBIT, PAIRS], [-TBIT, 2], [1, w]],
                    )
                else:
                    in_ap = bass.AP(src, base + e0 * P, [[1, w * P]])
                sync.dma_start(out=tile_view(i, w), in_=in_ap).then_inc(
                    load_sem, 16
                )

        @block.scalar
        def _(scalar):
            for i, (kind, base, e0, w) in enumerate(units):
                scalar.wait_ge(load_sem, 16 * (i + 1))
                if kind == "s":
                    out_ap = bass.AP(dst, base + e0, [[TBIT, P], [1, w]])
                else:
                    out_ap = bass.AP(dst, base + e0 * P, [[1, w * P]])
                scalar.dma_start(
                    out=out_ap, in_=tile_view(i, w)
                ).then_inc(store_sem, 16)
            scalar.wait_ge(store_sem, 16 * n)


def _build_nc(inplace):
    nc = bass.Bass(target_bir_lowering=False)
    out = nc.dram_tensor("out", (ROWS, DIM), mybir.dt.float32, kind="ExternalOutput")
    if inplace:
        _emit_bounce(nc, out, out, _units(inplace=True))
    else:
        st = nc.dram_tensor(
            "state", (ROWS, DIM), mybir.dt.float32, kind="ExternalInput"
        )
        _emit_bounce(nc, st, out, _units(inplace=False))
    if not nc.is_finalized():
        nc.finalize()
    return nc


def _get_nc(inplace):
    key = ("ip" if inplace else "fc",)
    if key not in _cache:
        _cache[key] = _build_nc(inplace)
    return _cache[key]


def _run_donated(nc, state):
    """Run `nc` via PJRT shard_map over 8 cores, donating the input state as
    the initial content of the (aliased) output buffer — the same donation
    mechanism run_bass_via_pjrt uses for its zero-filled outputs."""
    import jax
    from jax.experimental.shard_map import shard_map
    from jax.sharding import Mesh, PartitionSpec

    from concourse.bass2jax import (
        _bass_exec_p,
        install_neuronx_cc_hook,
        partition_id_tensor,
    )

    install_neuronx_cc_hook()

    out_names, out_avals = [], []
    for alloc in nc.m.functions[0].allocations:
        if (
            isinstance(alloc, mybir.MemoryLocationSet)
            and alloc.kind == "ExternalOutput"
        ):
            out_names.append(alloc.memorylocations[0].name)
            out_avals.append(
                jax.core.ShapedArray(
                    tuple(alloc.tensor_shape), mybir.dt.np(alloc.dtype)
                )
            )
    partition_name = nc.partition_id_tensor.name if nc.partition_id_tensor else None
    in_names = list(out_names)
    if partition_name is not None:
        in_names.append(partition_name)

    if "donated_fn" not in _cache:

        def _body(buf):
            operands = [buf]
            if partition_name is not None:
                operands.append(partition_id_tensor())
            outs = _bass_exec_p.bind(
                *operands,
                out_avals=tuple(out_avals),
                in_names=tuple(in_names),
                out_names=tuple(out_names),
                lowering_input_output_aliases=(),
                sim_require_finite=True,
                sim_require_nnan=True,
                nc=nc,
            )
            return outs[0]

        devices = jax.devices()[:N_CORES]
        mesh = Mesh(np.asarray(devices), ("core",))
        _cache["donated_fn"] = jax.jit(
            shard_map(
                _body,
                mesh=mesh,
                in_specs=(PartitionSpec("core"),),
                out_specs=PartitionSpec("core"),
                check_rep=False,
            ),
            donate_argnums=(0,),
            keep_unused=True,
        )

    out = _cache["donated_fn"](state)
    return np.asarray(out)


def _sample_ok(state, out, rng, k=2048):
    """Spot-check out[b, j] == state[b, j ^ (1<<13) if bit20(j) else j]."""
    b = rng.integers(0, BATCH, size=k)
    j = rng.integers(0, DIM, size=k)
    src = np.where((j >> C2) & 1 == 1, j ^ TBIT, j)
    return np.array_equal(out[b, j], state[b, src])


def kernel(state, control=3, target=10, num_qubits=24, **_):
    state = np.ascontiguousarray(np.asarray(state, dtype=np.float32))
    assert state.shape == (BATCH, DIM), state.shape
    assert int(control) == 3 and int(target) == 10 and int(num_qubits) == 24

    rng = np.random.default_rng(0)
    try:
        out = _run_donated(_get_nc(inplace=True), state)
        if _sample_ok(state, out, rng):
            return out
    except Exception:
        # Retry once with a fresh jit: a transient dispatch failure before
        # any device execution is cheap to retry; a second failure means the
        # aliasing mechanism is broken here -> full-copy.
        _cache.pop("donated_fn", None)
        try:
            out = _run_donated(_get_nc(inplace=True), state)
            if _sample_ok(state, out, rng):
                return out
        except Exception:
            pass

    # Fallback: full-copy kernel through run_bass_kernel_spmd.
    nc = _get_nc(inplace=False)
    in_maps = [{"state": state[c * ROWS : (c + 1) * ROWS]} for c in range(N_CORES)]
    res = run_bass_kernel_spmd(nc, in_maps, core_ids=list(range(N_CORES)))
    return np.concatenate([r["out"] for r in res.results], axis=0)


# revision 7
# speedup vs baseline: 1.5450x; 1.5450x over previous
"""CNOT permutation kernel for Trainium2 (Bass), 8-core data parallel.

Problem (hardcoded from spec): state (16, 2**24) f32, control=3, target=10,
num_qubits=24.  With c2 = 24-3-1 = 20 and t2 = 24-10-1 = 13:

    out[b, j] = state[b, j ^ (1<<13)]  if (j >> 20) & 1 else state[b, j]

Pure data movement.  Viewing the per-core shard flat (row stride 2**24 is a
multiple of the 2**21 control-bit period, so both rows fuse) as
[blk:16, ctrl:2, c:64, d:2, e:8192]:

    out[blk, 0, c, d, e] = in[blk, 0, c, d, e]      (identity half)
    out[blk, 1, c, d, e] = in[blk, 1, c, 1-d, e]    (swap 8192-elem chunk pairs)

Sharding: batch axis, 2 rows per core (pure data parallel).

Device kernels are HBM->SBUF->HBM bounces (direct DRAM->DRAM DMA serializes
read+write inside each SDMA engine; split load/store descriptors stream at
the SBUF-AXI fabric rate ~435 GB/s/core).  Loads issue on the Sync HWDGE
ring, stores on the Scalar ring; the 16 SDMA engines round-robin between
rings at packet granularity so both HBM directions stay busy.

Pipeline structure (this is where the previous version lost ~60 us/core):
the work is cut into variable-size units -- small at the head (first store
starts after a ~1 MiB load instead of 4 MiB) and geometrically shrinking at
the tail (the final load->store chain, which nothing can overlap, is 0.25
MiB instead of 4 MiB).  Units are one load + one CONTIGUOUS store each: the
chunk-pair swap is folded into the load AP (middle dim with stride -TBIT),
so HBM writes are fully sequential.  SBUF ring of NSLOT 32 KiB/partition
slots, two semaphores (RAW: store waits its unit's load; WAR: load waits
the store that previously used its slot).

1. In-place (default): a single DRAM tensor, pre-initialized with the input
   shard by donating it as the PJRT output buffer (the same donation
   mechanism run_bass_via_pjrt uses for its zero-filled outputs).  Only the
   control-bit=1 half is touched: 64 MiB read + 64 MiB write per core.  The
   identity half is never moved at all.
2. Full-copy (fallback): separate in/out tensors, 128+128 MiB per core.

kernel() runs the in-place path and sample-checks the permutation against
the host input; if the donation aliasing ever fails to hold (output buffer
not seeded with the input), it falls back to the full-copy path.
"""

import numpy as np

import concourse.bass as bass
import concourse.mybir as mybir
from concourse.bass_utils import run_bass_kernel_spmd

NUM_QUBITS = 24
DIM = 1 << NUM_QUBITS
BATCH = 16
N_CORES = 8
ROWS = BATCH // N_CORES  # 2 rows per core
C2 = NUM_QUBITS - 3 - 1  # 20
T2 = NUM_QUBITS - 10 - 1  # 13
CBIT = 1 << C2  # 1048576 elements (4 MiB)
TBIT = 1 << T2  # 8192 elements (32 KiB)
BLK = 2 * CBIT  # control-bit period
NBLK = ROWS * DIM // BLK  # 16 blocks in the fused per-core space

P = 128
PAIRS = CBIT // (2 * TBIT)  # 64 chunk pairs per 4 MiB region

# Units are e-axis slices (each spans all 128 partitions).  BODY_W elems /
# partition per unit in steady state; the first region ramps up and the
# last ramps down so the final un-overlappable load->store chain is tiny.
BODY_W = 4096  # 2 MiB units
NSLOT = 12  # SBUF ring: 12 x 16 KiB/partition = 192 KiB of ~208 usable
FREE = BODY_W
HEAD_SPLIT = [2048, 2048, 4096]
TAIL_SPLIT = [4096, 2048, 1024, 512, 512]

_cache = {}


def _region_units(kind, base, splits):
    out, e0 = [], 0
    for w in splits:
        out.append((kind, base, e0, w))
        e0 += w
    assert e0 == TBIT
    return out


def _units(inplace):
    """Yield (kind, region_base, e0, w): the unit covers the e in [e0,e0+w)
    slice of all 128 TBIT-chunks of its 4 MiB region.  kind 's'=swap,
    'i'=identity."""
    regions = []
    for b in range(NBLK):
        if not inplace:
            regions.append(("i", b * BLK))
        regions.append(("s", b * BLK + CBIT))
    body = [BODY_W] * (TBIT // BODY_W)
    units = []
    for r, (kind, base) in enumerate(regions):
        if r == 0:
            splits = HEAD_SPLIT
        elif r == len(regions) - 1:
            splits = TAIL_SPLIT
        else:
            splits = body
        units.extend(_region_units(kind, base, splits))
    return units


def _emit_bounce(nc, src, dst, units):
    """Loads on sync / stores on scalar, NSLOT-deep pipeline over units."""
    n = len(units)
    with (
        nc.sbuf_tensor("tiles", [P, NSLOT * FREE], mybir.dt.float32) as tiles,
        nc.semaphore("load_sem") as load_sem,
        nc.semaphore("store_sem") as store_sem,
        nc.Block() as block,
    ):

        def tile_view(i, w):
            s = (i % NSLOT) * FREE
            return tiles[:, s : s + w]

        @block.sync
        def _(sync):
            for i, (kind, base, e0, w) in enumerate(units):
                if i >= NSLOT:
                    sync.wait_ge(store_sem, 16 * (i - NSLOT + 1))
                if kind == "s":
                    # partition p = (c, j) reads chunk (c, 1-j)'s e-slice
                    in_ap = bass.AP(
                        src,
                        base + TBIT + e0,
                        [[2 * TBIT, PAIRS], [-TBIT, 2], [1, w]],
                    )
                else:
                    in_ap = bass.AP(src, base + e0 * P, [[1, w * P]])
                sync.dma_start(out=tile_view(i, w), in_=in_ap).then_inc(
                    load_sem, 16
                )

        @block.scalar
        def _(scalar):
            for i, (kind, base, e0, w) in enumerate(units):
                scalar.wait_ge(load_sem, 16 * (i + 1))
                if kind == "s":
                    out_ap = bass.AP(dst, base + e0, [[TBIT, P], [1, w]])
                else:
                    out_ap = bass.AP(dst, base + e0 * P, [[1, w * P]])
                scalar.dma_start(
                    out=out_ap, in_=tile_view(i, w)
                ).then_inc(store_sem, 16)
            scalar.wait_ge(store_sem, 16 * n)


def _build_nc(inplace):
    nc = bass.Bass(target_bir_lowering=False)
    out = nc.dram_tensor("out", (ROWS, DIM), mybir.dt.float32, kind="ExternalOutput")
    if inplace:
        _emit_bounce(nc, out, out, _units(inplace=True))
    else:
        st = nc.dram_tensor(
            "state", (ROWS, DIM), mybir.dt.float32, kind="ExternalInput"
        )
        _emit_bounce(nc, st, out, _units(inplace=False))
    if not nc.is_finalized():
        nc.finalize()
    return nc


def _get_nc(inplace):
    key = ("ip" if inplace else "fc",)
    if key not in _cache:
        _cache[key] = _build_nc(inplace)
    return _cache[key]


def _run_donated(nc, state):
    """Run `nc` via PJRT shard_map over 8 cores, donating the input state as
    the initial content of the (aliased) output buffer — the same donation
    mechanism run_bass_via_pjrt uses for its zero-filled outputs."""
    import jax
    from jax.experimental.shard_map import shard_map
    from jax.sharding import Mesh, PartitionSpec

    from concourse.bass2jax import (
        _bass_exec_p,
        install_neuronx_cc_hook,
        partition_id_tensor,
    )

    install_neuronx_cc_hook()

    out_names, out_avals = [], []
    for alloc in nc.m.functions[0].allocations:
        if (
            isinstance(alloc, mybir.MemoryLocationSet)
            and alloc.kind == "ExternalOutput"
        ):
            out_names.append(alloc.memorylocations[0].name)
            out_avals.append(
                jax.core.ShapedArray(
                    tuple(alloc.tensor_shape), mybir.dt.np(alloc.dtype)
                )
            )
    partition_name = nc.partition_id_tensor.name if nc.partition_id_tensor else None
    in_names = list(out_names)
    if partition_name is not None:
        in_names.append(partition_name)

    if "donated_fn" not in _cache:

        def _body(buf):
            operands = [buf]
            if partition_name is not None:
                operands.append(partition_id_tensor())
            outs = _bass_exec_p.bind(
                *operands,
                out_avals=tuple(out_avals),
                in_names=tuple(in_names),
                out_names=tuple(out_names),
                lowering_input_output_aliases=(),
                sim_require_finite=True,
                sim_require_nnan=True,
                nc=nc,
            )
            return outs[0]

        devices = jax.devices()[:N_CORES]
        mesh = Mesh(np.asarray(devices), ("core",))
        _cache["donated_fn"] = jax.jit(
            shard_map(
                _body,
                mesh=mesh,
                in_specs=(PartitionSpec("core"),),
                out_specs=PartitionSpec("core"),
                check_rep=False,
            ),
            donate_argnums=(0,),
            keep_unused=True,
        )

    out = _cache["donated_fn"](state)
    return np.asarray(out)


def _sample_ok(state, out, rng, k=2048):
    """Spot-check out[b, j] == state[b, j ^ (1<<13) if bit20(j) else j]."""
    b = rng.integers(0, BATCH, size=k)
    j = rng.integers(0, DIM, size=k)
    src = np.where((j >> C2) & 1 == 1, j ^ TBIT, j)
    return np.array_equal(out[b, j], state[b, src])


def kernel(state, control=3, target=10, num_qubits=24, **_):
    state = np.ascontiguousarray(np.asarray(state, dtype=np.float32))
    assert state.shape == (BATCH, DIM), state.shape
    assert int(control) == 3 and int(target) == 10 and int(num_qubits) == 24

    rng = np.random.default_rng(0)
    try:
        out = _run_donated(_get_nc(inplace=True), state)
        if _sample_ok(state, out, rng):
            return out
    except Exception:
        # Retry once with a fresh jit: a transient dispatch failure before
        # any device execution is cheap to retry; a second failure means the
        # aliasing mechanism is broken here -> full-copy.
        _cache.pop("donated_fn", None)
        try:
            out = _run_donated(_get_nc(inplace=True), state)
            if _sample_ok(state, out, rng):
                return out
        except Exception:
            pass

    # Fallback: full-copy kernel through run_bass_kernel_spmd.
    nc = _get_nc(inplace=False)
    in_maps = [{"state": state[c * ROWS : (c + 1) * ROWS]} for c in range(N_CORES)]
    res = run_bass_kernel_spmd(nc, in_maps, core_ids=list(range(N_CORES)))
    return np.concatenate([r["out"] for r in res.results], axis=0)


# revision 8
# speedup vs baseline: 1.6590x; 1.0737x over previous
"""CNOT permutation kernel for Trainium2 (Bass), 8-core data parallel.

Problem (hardcoded from spec): state (16, 2**24) f32, control=3, target=10,
num_qubits=24.  With c2 = 24-3-1 = 20 and t2 = 24-10-1 = 13:

    out[b, j] = state[b, j ^ (1<<13)]  if (j >> 20) & 1 else state[b, j]

Pure data movement.  Viewing the per-core shard flat (row stride 2**24 is a
multiple of the 2**21 control-bit period, so both rows fuse) as
[blk:16, ctrl:2, c:64, d:2, e:8192]:

    out[blk, 0, c, d, e] = in[blk, 0, c, d, e]      (identity half)
    out[blk, 1, c, d, e] = in[blk, 1, c, 1-d, e]    (swap 8192-elem chunk pairs)

Sharding: batch axis, 2 rows per core (pure data parallel).

Device kernels are HBM->SBUF->HBM bounces (direct DRAM->DRAM DMA serializes
read+write inside each SDMA engine; split load/store descriptors stream at
the SBUF-AXI fabric rate ~435 GB/s/core).  Loads issue on the Sync HWDGE
ring, stores on the Scalar ring; the 16 SDMA engines round-robin between
rings at packet granularity so both HBM directions stay busy.

Pipeline structure (this is where the previous version lost ~60 us/core):
the work is cut into variable-size units -- small at the head (first store
starts after a ~1 MiB load instead of 4 MiB) and geometrically shrinking at
the tail (the final load->store chain, which nothing can overlap, is 0.25
MiB instead of 4 MiB).  Units are one load + one CONTIGUOUS store each: the
chunk-pair swap is folded into the load AP (middle dim with stride -TBIT),
so HBM writes are fully sequential.  SBUF ring of NSLOT 32 KiB/partition
slots, two semaphores (RAW: store waits its unit's load; WAR: load waits
the store that previously used its slot).

1. In-place (default): a single DRAM tensor, pre-initialized with the input
   shard by donating it as the PJRT output buffer (the same donation
   mechanism run_bass_via_pjrt uses for its zero-filled outputs).  Only the
   control-bit=1 half is touched: 64 MiB read + 64 MiB write per core.  The
   identity half is never moved at all.
2. Full-copy (fallback): separate in/out tensors, 128+128 MiB per core.

kernel() runs the in-place path and sample-checks the permutation against
the host input; if the donation aliasing ever fails to hold (output buffer
not seeded with the input), it falls back to the full-copy path.
"""

import numpy as np

import concourse.bass as bass
import concourse.mybir as mybir
from concourse.bass_utils import run_bass_kernel_spmd

NUM_QUBITS = 24
DIM = 1 << NUM_QUBITS
BATCH = 16
N_CORES = 8
ROWS = BATCH // N_CORES  # 2 rows per core
C2 = NUM_QUBITS - 3 - 1  # 20
T2 = NUM_QUBITS - 10 - 1  # 13
CBIT = 1 << C2  # 1048576 elements (4 MiB)
TBIT = 1 << T2  # 8192 elements (32 KiB)
BLK = 2 * CBIT  # control-bit period
NBLK = ROWS * DIM // BLK  # 16 blocks in the fused per-core space

P = 128
PAIRS = CBIT // (2 * TBIT)  # 64 chunk pairs per 4 MiB region

# Units are e-axis slices (each spans all 128 partitions).  BODY_W elems /
# partition per unit in steady state; the first region ramps up and the
# last ramps down so the final un-overlappable load->store chain is tiny.
BODY_W = 8192  # 4 MiB units: body stores collapse to one contiguous run
NSLOT = 6  # SBUF ring: 6 x 32 KiB/partition = 192 KiB of ~208 usable
FREE = BODY_W
HEAD_SPLIT = [2048, 2048, 4096]
TAIL_SPLIT = [4096, 2048, 1024, 512, 512]

_cache = {}


def _region_units(kind, base, splits):
    out, e0 = [], 0
    for w in splits:
        out.append((kind, base, e0, w))
        e0 += w
    assert e0 == TBIT
    return out


def _units(inplace):
    """Yield (kind, region_base, e0, w): the unit covers the e in [e0,e0+w)
    slice of all 128 TBIT-chunks of its 4 MiB region.  kind 's'=swap,
    'i'=identity."""
    regions = []
    for b in range(NBLK):
        if not inplace:
            regions.append(("i", b * BLK))
        regions.append(("s", b * BLK + CBIT))
    body = [BODY_W] * (TBIT // BODY_W)
    units = []
    for r, (kind, base) in enumerate(regions):
        if r == 0:
            splits = HEAD_SPLIT
        elif r == len(regions) - 1:
            splits = TAIL_SPLIT
        else:
            splits = body
        units.extend(_region_units(kind, base, splits))
    return units


def _emit_bounce(nc, src, dst, units):
    """Loads on sync / stores on scalar, NSLOT-deep pipeline over units."""
    n = len(units)
    with (
        nc.sbuf_tensor("tiles", [P, NSLOT * FREE], mybir.dt.float32) as tiles,
        nc.semaphore("load_sem") as load_sem,
        nc.semaphore("store_sem") as store_sem,
        nc.Block() as block,
    ):

        def tile_view(i, w):
            s = (i % NSLOT) * FREE
            return tiles[:, s : s + w]

        @block.sync
        def _(sync):
            for i, (kind, base, e0, w) in enumerate(units):
                if i >= NSLOT:
                    sync.wait_ge(store_sem, 16 * (i - NSLOT + 1))
                if kind == "s":
                    # partition p = (c, j) reads chunk (c, 1-j)'s e-slice
                    in_ap = bass.AP(
                        src,
                        base + TBIT + e0,
                        [[2 * TBIT, PAIRS], [-TBIT, 2], [1, w]],
                    )
                else:
                    in_ap = bass.AP(src, base + e0 * P, [[1, w * P]])
                sync.dma_start(out=tile_view(i, w), in_=in_ap).then_inc(
                    load_sem, 16
                )

        @block.scalar
        def _(scalar):
            for i, (kind, base, e0, w) in enumerate(units):
                scalar.wait_ge(load_sem, 16 * (i + 1))
                if kind == "s":
                    out_ap = bass.AP(dst, base + e0, [[TBIT, P], [1, w]])
                else:
                    out_ap = bass.AP(dst, base + e0 * P, [[1, w * P]])
                scalar.dma_start(
                    out=out_ap, in_=tile_view(i, w)
                ).then_inc(store_sem, 16)
            scalar.wait_ge(store_sem, 16 * n)


def _build_nc(inplace):
    nc = bass.Bass(target_bir_lowering=False)
    out = nc.dram_tensor("out", (ROWS, DIM), mybir.dt.float32, kind="ExternalOutput")
    if inplace:
        _emit_bounce(nc, out, out, _units(inplace=True))
    else:
        st = nc.dram_tensor(
            "state", (ROWS, DIM), mybir.dt.float32, kind="ExternalInput"
        )
        _emit_bounce(nc, st, out, _units(inplace=False))
    if not nc.is_finalized():
        nc.finalize()
    return nc


def _get_nc(inplace):
    key = ("ip" if inplace else "fc",)
    if key not in _cache:
        _cache[key] = _build_nc(inplace)
    return _cache[key]


def _run_donated(nc, state):
    """Run `nc` via PJRT shard_map over 8 cores, donating the input state as
    the initial content of the (aliased) output buffer — the same donation
    mechanism run_bass_via_pjrt uses for its zero-filled outputs."""
    import jax
    from jax.experimental.shard_map import shard_map
    from jax.sharding import Mesh, PartitionSpec

    from concourse.bass2jax import (
        _bass_exec_p,
        install_neuronx_cc_hook,
        partition_id_tensor,
    )

    install_neuronx_cc_hook()

    out_names, out_avals = [], []
    for alloc in nc.m.functions[0].allocations:
        if (
            isinstance(alloc, mybir.MemoryLocationSet)
            and alloc.kind == "ExternalOutput"
        ):
            out_names.append(alloc.memorylocations[0].name)
            out_avals.append(
                jax.core.ShapedArray(
                    tuple(alloc.tensor_shape), mybir.dt.np(alloc.dtype)
                )
            )
    partition_name = nc.partition_id_tensor.name if nc.partition_id_tensor else None
    in_names = list(out_names)
    if partition_name is not None:
        in_names.append(partition_name)

    if "donated_fn" not in _cache:

        def _body(buf):
            operands = [buf]
            if partition_name is not None:
                operands.append(partition_id_tensor())
            outs = _bass_exec_p.bind(
                *operands,
                out_avals=tuple(out_avals),
                in_names=tuple(in_names),
                out_names=tuple(out_names),
                lowering_input_output_aliases=(),
                sim_require_finite=True,
                sim_require_nnan=True,
                nc=nc,
            )
            return outs[0]

        devices = jax.devices()[:N_CORES]
        mesh = Mesh(np.asarray(devices), ("core",))
        _cache["donated_fn"] = jax.jit(
            shard_map(
                _body,
                mesh=mesh,
                in_specs=(PartitionSpec("core"),),
                out_specs=PartitionSpec("core"),
                check_rep=False,
            ),
            donate_argnums=(0,),
            keep_unused=True,
        )

    out = _cache["donated_fn"](state)
    return np.asarray(out)


def _sample_ok(state, out, rng, k=2048):
    """Spot-check out[b, j] == state[b, j ^ (1<<13) if bit20(j) else j]."""
    b = rng.integers(0, BATCH, size=k)
    j = rng.integers(0, DIM, size=k)
    src = np.where((j >> C2) & 1 == 1, j ^ TBIT, j)
    return np.array_equal(out[b, j], state[b, src])


def kernel(state, control=3, target=10, num_qubits=24, **_):
    state = np.ascontiguousarray(np.asarray(state, dtype=np.float32))
    assert state.shape == (BATCH, DIM), state.shape
    assert int(control) == 3 and int(target) == 10 and int(num_qubits) == 24

    rng = np.random.default_rng(0)
    try:
        out = _run_donated(_get_nc(inplace=True), state)
        if _sample_ok(state, out, rng):
            return out
    except Exception:
        # Retry once with a fresh jit: a transient dispatch failure before
        # any device execution is cheap to retry; a second failure means the
        # aliasing mechanism is broken here -> full-copy.
        _cache.pop("donated_fn", None)
        try:
            out = _run_donated(_get_nc(inplace=True), state)
            if _sample_ok(state, out, rng):
                return out
        except Exception:
            pass

    # Fallback: full-copy kernel through run_bass_kernel_spmd.
    nc = _get_nc(inplace=False)
    in_maps = [{"state": state[c * ROWS : (c + 1) * ROWS]} for c in range(N_CORES)]
    res = run_bass_kernel_spmd(nc, in_maps, core_ids=list(range(N_CORES)))
    return np.concatenate([r["out"] for r in res.results], axis=0)
